# revision 6
# baseline (speedup 1.0000x reference)
"""Butterfly multiply (n=4096, 12 stages, increasing stride) on 8 Trainium2
NeuronCores.

Math: the 12 butterfly stages factor into
  out = P^T-scatter( B-blockdiag @ P-permute( A-blockdiag @ x^T ) )
where stages 0..6 (strides 1..64) compose into 32 dense 128x128 matrices A_o
acting within 128-aligned blocks, and stages 7..11 (strides 128..2048) compose
into 128 dense 32x32 matrices C_i acting across blocks at fixed within-block
index.  Both are composed on the host from the (tiny) twiddle input; the heavy
data (x: 128 MiB) runs through two TensorEngine matmul passes per core.

Sharding: batch 8192 split across 8 cores (data parallel), twiddle-derived
matrices replicated.
"""

import os
import sys
import numpy as np

LOG_N = 12
N = 4096
BATCH = 8192
N_CORES = 8
B_CORE = BATCH // N_CORES  # 1024 rows per core

# compute dtype: "fp32" (safe, PE quarter-rate) or "fp16" (fast, ~1e-3 rel err)
COMPUTE = os.environ.get("BUTTERFLY_COMPUTE", "fp32")
BC = 128  # batch chunk rows processed per pipeline step


def _compose_matrices(twiddle):
    """Compose stages 0..6 -> A (32,128,128) and stages 7..11 -> C (128,32,32),
    in float64."""
    tw = np.asarray(twiddle)[0, 0].astype(np.float64)  # (12, 2048, 2, 2)

    A = np.zeros((32, 128, 128))
    A[:, np.arange(128), np.arange(128)] = 1.0
    for idx in range(7):
        s = 1 << idx
        Ar = A.reshape(32, 128 // (2 * s), 2, s, 128)  # (o, dl, k, j, i_in)
        o = np.arange(32)[:, None, None]
        dl = np.arange(128 // (2 * s))[None, :, None]
        j = np.arange(s)[None, None, :]
        m = (o * (64 // s) + dl) * s + j
        t = tw[idx, m]  # (32, dl, j, 2, 2)
        x0, x1 = Ar[:, :, 0], Ar[:, :, 1]
        new0 = t[..., 0, 0:1] * x0 + t[..., 0, 1:2] * x1
        new1 = t[..., 1, 0:1] * x0 + t[..., 1, 1:2] * x1
        A = np.stack([new0, new1], axis=2).reshape(32, 128, 128)

    C = np.zeros((128, 32, 32))
    C[:, np.arange(32), np.arange(32)] = 1.0
    for idx in range(7, 12):
        s = 1 << idx
        sp = s // 128
        Cr = C.reshape(128, 32 // (2 * sp), 2, sp, 32)  # (i, dl, k, ol, o_in)
        i = np.arange(128)[None, None, :]
        dl = np.arange(32 // (2 * sp))[:, None, None]
        ol = np.arange(sp)[None, :, None]
        m = dl * (128 * sp) + 128 * ol + i  # (dl, ol, i)
        t = np.moveaxis(tw[idx, m], 2, 0)  # (i, dl, ol, 2, 2)
        x0, x1 = Cr[:, :, 0], Cr[:, :, 1]
        new0 = t[..., 0, 0:1] * x0 + t[..., 0, 1:2] * x1
        new1 = t[..., 1, 0:1] * x0 + t[..., 1, 1:2] * x1
        C = np.stack([new0, new1], axis=2).reshape(128, 32, 32)

    # AT[o] = A[o].T  (lhsT layout for pass A)
    AT = np.ascontiguousarray(np.transpose(A, (0, 2, 1)))
    # R[tau][u=(a*32+o_in), v=(o_out*4+a)] = C[4*tau+a][o_out, o_in]
    R = np.zeros((32, 128, 128))
    for tau in range(32):
        for a in range(4):
            R[tau, a * 32:(a + 1) * 32, a::4] = C[4 * tau + a].T
    return AT, R


def _build_program(np_dt, mybir_dt, b_core=B_CORE):
    """Trace + compile the per-core Bass program. Returns nc."""
    import concourse.bacc as bacc
    import concourse.tile as tile
    import concourse.mybir as mybir
    from contextlib import ExitStack

    f32 = mybir.dt.float32
    dt = mybir_dt

    nc = bacc.Bacc(
        "TRN2",
        target_bir_lowering=False,
        debug=False,
        enable_asserts=False,
        num_devices=1,
    )
    x_ap = nc.dram_tensor("x", (b_core, N), dt, kind="ExternalInput").ap()
    # AT/R shipped pre-arranged as [k, o*128+m] so the load is a plain 2D copy
    at_ap = nc.dram_tensor("AT", (128, 32 * 128), dt, kind="ExternalInput").ap()
    r_ap = nc.dram_tensor("R", (128, 32 * 128), dt, kind="ExternalInput").ap()
    id_ap = nc.dram_tensor("ident", (128, 128), dt, kind="ExternalInput").ap()
    y_ap = nc.dram_tensor("y", (b_core, N), dt, kind="ExternalOutput").ap()

    n_chunks = b_core // BC

    with tile.TileContext(nc) as tc, ExitStack() as ctx:
        wpool = ctx.enter_context(tc.tile_pool(name="weights", bufs=1))
        xn_pool = ctx.enter_context(tc.tile_pool(name="xn", bufs=2))
        xT_pool = ctx.enter_context(tc.tile_pool(name="xT", bufs=2))
        y1_pool = ctx.enter_context(tc.tile_pool(name="y1", bufs=2))
        z_pool = ctx.enter_context(tc.tile_pool(name="z", bufs=2))
        out_pool = ctx.enter_context(tc.tile_pool(name="outb", bufs=2))
        psT_pool = ctx.enter_context(tc.tile_pool(name="psT", bufs=2, space="PSUM"))
        psA_pool = ctx.enter_context(tc.tile_pool(name="psA", bufs=2, space="PSUM"))
        psB_pool = ctx.enter_context(tc.tile_pool(name="psB", bufs=2, space="PSUM"))

        ATw = wpool.tile([128, 32 * 128], dt, tag="ATw")
        Rw = wpool.tile([128, 32 * 128], dt, tag="Rw")
        identw = wpool.tile([128, 128], dt, tag="identw")
        nc.sync.dma_start(ATw[:], at_ap)
        nc.sync.dma_start(Rw[:], r_ap)
        nc.sync.dma_start(identw[:], id_ap)

        for cc in range(n_chunks):
            # 1. load natural-layout chunk [BC x N]
            xn = xn_pool.tile([BC, N], dt, tag="xn")
            nc.sync.dma_start(xn[:], x_ap[cc * BC:(cc + 1) * BC, :])

            # 2. PE transpose to n-on-partitions + copy PSUM->SBUF
            xT = xT_pool.tile([128, 32 * BC], dt, tag="xT")
            for g in range(8):
                psT = psT_pool.tile([128, 4 * BC], f32, tag="psT")
                for jj in range(4):
                    o = 4 * g + jj
                    nc.tensor.transpose(
                        psT[:, jj * BC:(jj + 1) * BC],
                        xn[:, o * 128:(o + 1) * 128],
                        identw[:],
                    )
                nc.any.tensor_copy(xT[:, g * 4 * BC:(g + 1) * 4 * BC], psT[:])

            # 3. pass A: y1[(o,i'), b] = sum_i A_o[i', i] xT[(o,i), b]
            y1 = y1_pool.tile([128, 32 * BC], dt, tag="y1")
            for g in range(8):
                psA = psA_pool.tile([128, 4 * BC], f32, tag="psA")
                for jj in range(4):
                    o = 4 * g + jj
                    nc.tensor.matmul(
                        psA[:, jj * BC:(jj + 1) * BC],
                        ATw[:, o * 128:(o + 1) * 128],
                        xT[:, o * BC:(o + 1) * BC],
                        start=True,
                        stop=True,
                    )
                nc.any.tensor_copy(y1[:, g * 4 * BC:(g + 1) * 4 * BC], psA[:])

            # 4. permute: z[(a*32+o), tau*BC+b] = y1[4*tau+a, o*BC+b]
            #    dst tile tau is a plain [128, BC] walk (partition u=a*32+o
            #    sequential); src iterates (a: partition, o: free, b: free) in
            #    the matching flat order.
            z = z_pool.tile([128, 32 * BC], dt, tag="z")
            for tau in range(32):
                nc.scalar.dma_start(
                    z[:, tau * BC:(tau + 1) * BC],
                    y1[4 * tau:4 * (tau + 1), :].rearrange("a (o b) -> a o b", b=BC),
                )

            # 5. pass B (data as stationary operand) + 6. scatter-copy
            outb = out_pool.tile([BC, N], dt, tag="outb")
            outb_re = outb[:].rearrange(
                "p (op gg tt a) -> gg p tt op a", op=32, gg=8, tt=4, a=4
            )
            for g in range(8):
                psB = psB_pool.tile([BC, 512], f32, tag="psB")
                for tt in range(4):
                    tau = 4 * g + tt
                    nc.tensor.matmul(
                        psB[:, tt * 128:(tt + 1) * 128],
                        z[:, tau * BC:(tau + 1) * BC],
                        Rw[:, tau * 128:(tau + 1) * 128],
                        start=True,
                        stop=True,
                    )
                nc.any.tensor_copy(
                    outb_re[g],
                    psB[:].rearrange("p (tt op a) -> p tt op a", tt=4, op=32, a=4),
                )

            # 7. contiguous store
            nc.sync.dma_start(y_ap[cc * BC:(cc + 1) * BC, :], outb[:])

    nc.compile()
    return nc


_CACHE = {}


def _get_program():
    import concourse.mybir as mybir

    key = COMPUTE
    if key not in _CACHE:
        if COMPUTE == "fp16":
            _CACHE[key] = (_build_program(np.float16, mybir.dt.float16), np.float16)
        else:
            _CACHE[key] = (_build_program(np.float32, mybir.dt.float32), np.float32)
    return _CACHE[key]


def run(x, twiddle, trace=False, trace_kwargs=None):
    """Run the butterfly kernel on 8 cores. Returns (out, BassKernelResults)."""
    from concourse.bass_utils import run_bass_kernel_spmd

    nc, np_dt = _get_program()

    AT, R = _compose_matrices(twiddle)
    # [o, k, m] -> [k, o*128+m] (matches the SBUF weight layout)
    ATd = np.ascontiguousarray(AT.transpose(1, 0, 2).reshape(128, 32 * 128)).astype(np_dt)
    Rd = np.ascontiguousarray(R.transpose(1, 0, 2).reshape(128, 32 * 128)).astype(np_dt)
    identd = np.eye(128, dtype=np_dt)

    x = np.asarray(x)
    in_dtype = x.dtype
    xd = np.ascontiguousarray(x.astype(np_dt))

    in_maps = []
    for c in range(N_CORES):
        in_maps.append(
            {
                "x": xd[c * B_CORE:(c + 1) * B_CORE],
                "AT": ATd,
                "R": Rd,
                "ident": identd,
            }
        )

    res = run_bass_kernel_spmd(
        nc,
        in_maps,
        core_ids=list(range(N_CORES)),
        trace=trace,
        **(trace_kwargs or {}),
    )
    out = np.concatenate([r["y"] for r in res.results], axis=0)
    return out.astype(in_dtype), res


def kernel(x, twiddle):
    out, _ = run(x, twiddle)
    return out


# revision 9
# speedup vs baseline: 1.2767x; 1.2767x over previous
"""Butterfly multiply (n=4096, 12 stages, increasing stride) on 8 Trainium2
NeuronCores.

Math: the 12 butterfly stages factor into
  out = P^T-scatter( B-blockdiag @ P-permute( A-blockdiag @ x^T ) )
where stages 0..6 (strides 1..64) compose into 32 dense 128x128 matrices A_o
acting within 128-aligned blocks, and stages 7..11 (strides 128..2048) compose
into 128 dense 32x32 matrices C_i acting across blocks at fixed within-block
index.  Both are composed on the host from the (tiny) twiddle input; the heavy
data (x: 128 MiB) runs through two TensorEngine matmul passes per core.

Sharding: batch 8192 split across 8 cores (data parallel), twiddle-derived
matrices replicated.
"""

import os
import sys
import numpy as np

LOG_N = 12
N = 4096
BATCH = 8192
N_CORES = 8
B_CORE = BATCH // N_CORES  # 1024 rows per core

# compute dtype: "fp32" (safe, PE quarter-rate) or "fp16" (fast, ~1e-3 rel err)
COMPUTE = os.environ.get("BUTTERFLY_COMPUTE", "fp32")
BC = 128   # inner chunk (transpose / pass-B granularity)
BCO = 256  # outer chunk (pass-A free dim, permute granularity)


def _compose_matrices(twiddle):
    """Compose stages 0..6 -> A (32,128,128) and stages 7..11 -> C (128,32,32),
    in float64."""
    tw = np.asarray(twiddle)[0, 0].astype(np.float64)  # (12, 2048, 2, 2)

    A = np.zeros((32, 128, 128))
    A[:, np.arange(128), np.arange(128)] = 1.0
    for idx in range(7):
        s = 1 << idx
        Ar = A.reshape(32, 128 // (2 * s), 2, s, 128)  # (o, dl, k, j, i_in)
        o = np.arange(32)[:, None, None]
        dl = np.arange(128 // (2 * s))[None, :, None]
        j = np.arange(s)[None, None, :]
        m = (o * (64 // s) + dl) * s + j
        t = tw[idx, m]  # (32, dl, j, 2, 2)
        x0, x1 = Ar[:, :, 0], Ar[:, :, 1]
        new0 = t[..., 0, 0:1] * x0 + t[..., 0, 1:2] * x1
        new1 = t[..., 1, 0:1] * x0 + t[..., 1, 1:2] * x1
        A = np.stack([new0, new1], axis=2).reshape(32, 128, 128)

    C = np.zeros((128, 32, 32))
    C[:, np.arange(32), np.arange(32)] = 1.0
    for idx in range(7, 12):
        s = 1 << idx
        sp = s // 128
        Cr = C.reshape(128, 32 // (2 * sp), 2, sp, 32)  # (i, dl, k, ol, o_in)
        i = np.arange(128)[None, None, :]
        dl = np.arange(32 // (2 * sp))[:, None, None]
        ol = np.arange(sp)[None, :, None]
        m = dl * (128 * sp) + 128 * ol + i  # (dl, ol, i)
        t = np.moveaxis(tw[idx, m], 2, 0)  # (i, dl, ol, 2, 2)
        x0, x1 = Cr[:, :, 0], Cr[:, :, 1]
        new0 = t[..., 0, 0:1] * x0 + t[..., 0, 1:2] * x1
        new1 = t[..., 1, 0:1] * x0 + t[..., 1, 1:2] * x1
        C = np.stack([new0, new1], axis=2).reshape(128, 32, 32)

    # AT[o] = A[o].T  (lhsT layout for pass A)
    AT = np.ascontiguousarray(np.transpose(A, (0, 2, 1)))
    # R[tau][u=(a*32+o_in), v=(o_out*4+a)] = C[4*tau+a][o_out, o_in]
    R = np.zeros((32, 128, 128))
    for tau in range(32):
        for a in range(4):
            R[tau, a * 32:(a + 1) * 32, a::4] = C[4 * tau + a].T
    return AT, R


def _build_program(np_dt, mybir_dt, b_core=B_CORE):
    """Trace + compile the per-core Bass program. Returns nc."""
    import concourse.bacc as bacc
    import concourse.tile as tile
    import concourse.mybir as mybir
    from contextlib import ExitStack

    f32 = mybir.dt.float32
    dt = mybir_dt

    nc = bacc.Bacc(
        "TRN2",
        target_bir_lowering=False,
        debug=False,
        enable_asserts=False,
        num_devices=1,
    )
    x_ap = nc.dram_tensor("x", (b_core, N), dt, kind="ExternalInput").ap()
    # AT/R shipped pre-arranged as [k, o*128+m] so the load is a plain 2D copy
    at_ap = nc.dram_tensor("AT", (128, 32 * 128), dt, kind="ExternalInput").ap()
    r_ap = nc.dram_tensor("R", (128, 32 * 128), dt, kind="ExternalInput").ap()
    id_ap = nc.dram_tensor("ident", (128, 128), dt, kind="ExternalInput").ap()
    y_ap = nc.dram_tensor("y", (b_core, N), dt, kind="ExternalOutput").ap()

    n_outer = b_core // BCO
    n_inner = BCO // BC

    with tile.TileContext(nc) as tc, ExitStack() as ctx:
        wpool = ctx.enter_context(tc.tile_pool(name="weights", bufs=1))
        xn_pool = ctx.enter_context(tc.tile_pool(name="xn", bufs=3))
        # xT doubles as y1: pass A's copy for block o overwrites the xT slice
        # of block o right after the matmul that consumed it.
        xT_pool = ctx.enter_context(tc.tile_pool(name="xT", bufs=1))
        z_pool = ctx.enter_context(tc.tile_pool(name="z", bufs=1))
        out_pool = ctx.enter_context(tc.tile_pool(name="outb", bufs=3))
        psT_pool = ctx.enter_context(tc.tile_pool(name="psT", bufs=2, space="PSUM"))
        psA_pool = ctx.enter_context(tc.tile_pool(name="psA", bufs=2, space="PSUM"))
        psB_pool = ctx.enter_context(tc.tile_pool(name="psB", bufs=2, space="PSUM"))

        ATw = wpool.tile([128, 32 * 128], dt, tag="ATw")
        Rw = wpool.tile([128, 32 * 128], dt, tag="Rw")
        identw = wpool.tile([128, 128], dt, tag="identw")
        nc.sync.dma_start(ATw[:], at_ap)
        nc.sync.dma_start(Rw[:], r_ap)
        nc.sync.dma_start(identw[:], id_ap)

        for cc in range(n_outer):
            # 2. PE transpose to n-on-partitions + copy PSUM->SBUF
            #    xT free layout: o*BCO + jc*BC + b
            xT = xT_pool.tile([128, 32 * BCO], dt, tag="xT")
            for jc in range(n_inner):
                # 1. load natural-layout chunk [BC x N]
                xn = xn_pool.tile([BC, N], dt, tag="xn")
                nc.sync.dma_start(
                    xn[:], x_ap[cc * BCO + jc * BC:cc * BCO + (jc + 1) * BC, :]
                )
                for g in range(8):
                    psT = psT_pool.tile([128, 4 * BC], f32, tag="psT")
                    for jj in range(4):
                        o = 4 * g + jj
                        nc.tensor.transpose(
                            psT[:, jj * BC:(jj + 1) * BC],
                            xn[:, o * 128:(o + 1) * 128],
                            identw[:],
                        )
                    # scatter the 4 transposed blocks to their o-slices
                    nc.any.tensor_copy(
                        xT[:].rearrange("p (o b) -> p o b", b=BCO)[
                            :, 4 * g:4 * g + 4, jc * BC:(jc + 1) * BC
                        ],
                        psT[:].rearrange("p (o b) -> p o b", o=4),
                    )

            # 3. pass A: y1[(o,i'), b] = sum_i A_o[i', i] xT[(o,i), b]
            #    y1 reuses the xT buffer (slice o dead once matmul o issued)
            y1 = xT
            for op in range(16):  # o-pairs -> one PSUM bank per pair
                psA = psA_pool.tile([128, 2 * BCO], f32, tag="psA")
                for jj in range(2):
                    o = 2 * op + jj
                    nc.tensor.matmul(
                        psA[:, jj * BCO:(jj + 1) * BCO],
                        ATw[:, o * 128:(o + 1) * 128],
                        xT[:, o * BCO:(o + 1) * BCO],
                        start=True,
                        stop=True,
                    )
                nc.any.tensor_copy(
                    y1[:, 2 * op * BCO:(2 * op + 2) * BCO], psA[:]
                )

            # 4. permute: z[(a*32+o), tau*BCO+b] = y1[4*tau+a, o*BCO+b]
            z = z_pool.tile([128, 32 * BCO], dt, tag="z")
            for tau in range(32):
                eng = nc.scalar if (tau % 2 == 0) else nc.sync
                eng.dma_start(
                    z[:, tau * BCO:(tau + 1) * BCO],
                    y1[4 * tau:4 * (tau + 1), :].rearrange("a (o b) -> a o b", b=BCO),
                )

            # 5. pass B (data as stationary operand) + 6. scatter-copy
            for jc in range(n_inner):
                outb = out_pool.tile([BC, N], dt, tag="outb")
                outb_re = outb[:].rearrange(
                    "p (op gg tt a) -> gg p tt op a", op=32, gg=8, tt=4, a=4
                )
                for g in range(8):
                    psB = psB_pool.tile([BC, 512], f32, tag="psB")
                    for tt in range(4):
                        tau = 4 * g + tt
                        nc.tensor.matmul(
                            psB[:, tt * 128:(tt + 1) * 128],
                            z[:, tau * BCO + jc * BC:tau * BCO + (jc + 1) * BC],
                            Rw[:, tau * 128:(tau + 1) * 128],
                            start=True,
                            stop=True,
                        )
                    nc.any.tensor_copy(
                        outb_re[g],
                        psB[:].rearrange("p (tt op a) -> p tt op a", tt=4, op=32, a=4),
                    )

                # 7. contiguous store
                nc.sync.dma_start(
                    y_ap[cc * BCO + jc * BC:cc * BCO + (jc + 1) * BC, :], outb[:]
                )

    nc.compile()
    return nc


_CACHE = {}


def _get_program():
    import concourse.mybir as mybir

    key = COMPUTE
    if key not in _CACHE:
        if COMPUTE == "fp16":
            _CACHE[key] = (_build_program(np.float16, mybir.dt.float16), np.float16)
        else:
            _CACHE[key] = (_build_program(np.float32, mybir.dt.float32), np.float32)
    return _CACHE[key]


def run(x, twiddle, trace=False, trace_kwargs=None):
    """Run the butterfly kernel on 8 cores. Returns (out, BassKernelResults)."""
    from concourse.bass_utils import run_bass_kernel_spmd

    nc, np_dt = _get_program()

    AT, R = _compose_matrices(twiddle)
    # [o, k, m] -> [k, o*128+m] (matches the SBUF weight layout)
    ATd = np.ascontiguousarray(AT.transpose(1, 0, 2).reshape(128, 32 * 128)).astype(np_dt)
    Rd = np.ascontiguousarray(R.transpose(1, 0, 2).reshape(128, 32 * 128)).astype(np_dt)
    identd = np.eye(128, dtype=np_dt)

    x = np.asarray(x)
    in_dtype = x.dtype
    xd = np.ascontiguousarray(x.astype(np_dt))

    in_maps = []
    for c in range(N_CORES):
        in_maps.append(
            {
                "x": xd[c * B_CORE:(c + 1) * B_CORE],
                "AT": ATd,
                "R": Rd,
                "ident": identd,
            }
        )

    res = run_bass_kernel_spmd(
        nc,
        in_maps,
        core_ids=list(range(N_CORES)),
        trace=trace,
        **(trace_kwargs or {}),
    )
    out = np.concatenate([r["y"] for r in res.results], axis=0)
    return out.astype(in_dtype), res


def kernel(x, twiddle):
    out, _ = run(x, twiddle)
    return out


# revision 10
# speedup vs baseline: 2.2579x; 1.7686x over previous
"""Butterfly multiply (n=4096, 12 stages, increasing stride) on 8 Trainium2
NeuronCores.

Math: the 12 butterfly stages factor into
  out = P^T-scatter( B-blockdiag @ P-permute( A-blockdiag @ x^T ) )
where stages 0..6 (strides 1..64) compose into 32 dense 128x128 matrices A_o
acting within 128-aligned blocks, and stages 7..11 (strides 128..2048) compose
into 128 dense 32x32 matrices C_i acting across blocks at fixed within-block
index.  Both are composed on the host from the (tiny) twiddle input; the heavy
data (x: 128 MiB) runs through two TensorEngine matmul passes per core.

Sharding: batch 8192 split across 8 cores (data parallel), twiddle-derived
matrices replicated.
"""

import os
import sys
import numpy as np

LOG_N = 12
N = 4096
BATCH = 8192
N_CORES = 8
B_CORE = BATCH // N_CORES  # 1024 rows per core

# compute dtype: "fp32" (safe, PE quarter-rate) or "fp16" (fast, ~1e-3 rel err)
COMPUTE = os.environ.get("BUTTERFLY_COMPUTE", "fp32")
BC = 128   # inner chunk (transpose / pass-B granularity)
BCO = 256  # outer chunk (pass-A free dim, permute granularity)


def _compose_matrices(twiddle):
    """Compose stages 0..6 -> A (32,128,128) and stages 7..11 -> C (128,32,32),
    in float64."""
    tw = np.asarray(twiddle)[0, 0].astype(np.float64)  # (12, 2048, 2, 2)

    A = np.zeros((32, 128, 128))
    A[:, np.arange(128), np.arange(128)] = 1.0
    for idx in range(7):
        s = 1 << idx
        Ar = A.reshape(32, 128 // (2 * s), 2, s, 128)  # (o, dl, k, j, i_in)
        o = np.arange(32)[:, None, None]
        dl = np.arange(128 // (2 * s))[None, :, None]
        j = np.arange(s)[None, None, :]
        m = (o * (64 // s) + dl) * s + j
        t = tw[idx, m]  # (32, dl, j, 2, 2)
        x0, x1 = Ar[:, :, 0], Ar[:, :, 1]
        new0 = t[..., 0, 0:1] * x0 + t[..., 0, 1:2] * x1
        new1 = t[..., 1, 0:1] * x0 + t[..., 1, 1:2] * x1
        A = np.stack([new0, new1], axis=2).reshape(32, 128, 128)

    C = np.zeros((128, 32, 32))
    C[:, np.arange(32), np.arange(32)] = 1.0
    for idx in range(7, 12):
        s = 1 << idx
        sp = s // 128
        Cr = C.reshape(128, 32 // (2 * sp), 2, sp, 32)  # (i, dl, k, ol, o_in)
        i = np.arange(128)[None, None, :]
        dl = np.arange(32 // (2 * sp))[:, None, None]
        ol = np.arange(sp)[None, :, None]
        m = dl * (128 * sp) + 128 * ol + i  # (dl, ol, i)
        t = np.moveaxis(tw[idx, m], 2, 0)  # (i, dl, ol, 2, 2)
        x0, x1 = Cr[:, :, 0], Cr[:, :, 1]
        new0 = t[..., 0, 0:1] * x0 + t[..., 0, 1:2] * x1
        new1 = t[..., 1, 0:1] * x0 + t[..., 1, 1:2] * x1
        C = np.stack([new0, new1], axis=2).reshape(128, 32, 32)

    # AT[o] = A[o].T  (lhsT layout for pass A)
    AT = np.ascontiguousarray(np.transpose(A, (0, 2, 1)))
    # R[tau][u=(a*32+o_in), v=(o_out*4+a)] = C[4*tau+a][o_out, o_in]
    R = np.zeros((32, 128, 128))
    for tau in range(32):
        for a in range(4):
            R[tau, a * 32:(a + 1) * 32, a::4] = C[4 * tau + a].T
    return AT, R


def _build_program(np_dt, mybir_dt, b_core=B_CORE):
    """Trace + compile the per-core Bass program. Returns nc."""
    import concourse.bacc as bacc
    import concourse.tile as tile
    import concourse.mybir as mybir
    from contextlib import ExitStack

    f32 = mybir.dt.float32
    dt = mybir_dt

    nc = bacc.Bacc(
        "TRN2",
        target_bir_lowering=False,
        debug=False,
        enable_asserts=False,
        num_devices=1,
    )
    x_ap = nc.dram_tensor("x", (b_core, N), dt, kind="ExternalInput").ap()
    # AT/R shipped pre-arranged as [k, o*128+m] so the load is a plain 2D copy
    at_ap = nc.dram_tensor("AT", (128, 32 * 128), dt, kind="ExternalInput").ap()
    r_ap = nc.dram_tensor("R", (128, 32 * 128), dt, kind="ExternalInput").ap()
    id_ap = nc.dram_tensor("ident", (128, 128), dt, kind="ExternalInput").ap()
    y_ap = nc.dram_tensor("y", (b_core, N), dt, kind="ExternalOutput").ap()

    n_outer = b_core // BCO
    n_inner = BCO // BC

    with tile.TileContext(nc) as tc, ExitStack() as ctx:
        wpool = ctx.enter_context(tc.tile_pool(name="weights", bufs=1))
        xn_pool = ctx.enter_context(tc.tile_pool(name="xn", bufs=3))
        # xT doubles as y1: pass A's copy for block o overwrites the xT slice
        # of block o right after the matmul that consumed it.
        xT_pool = ctx.enter_context(tc.tile_pool(name="xT", bufs=1))
        z_pool = ctx.enter_context(tc.tile_pool(name="z", bufs=1))
        out_pool = ctx.enter_context(tc.tile_pool(name="outb", bufs=3))
        psT_pool = ctx.enter_context(tc.tile_pool(name="psT", bufs=2, space="PSUM"))
        psA_pool = ctx.enter_context(tc.tile_pool(name="psA", bufs=2, space="PSUM"))
        psB_pool = ctx.enter_context(tc.tile_pool(name="psB", bufs=2, space="PSUM"))

        ATw = wpool.tile([128, 32 * 128], dt, tag="ATw")
        Rw = wpool.tile([128, 32 * 128], dt, tag="Rw")
        identw = wpool.tile([128, 128], dt, tag="identw")
        nc.sync.dma_start(ATw[:], at_ap)
        nc.sync.dma_start(Rw[:], r_ap)
        nc.sync.dma_start(identw[:], id_ap)

        for cc in range(n_outer):
            # 2. PE transpose to n-on-partitions + copy PSUM->SBUF
            #    xT free layout: o*BCO + jc*BC + b
            xT = xT_pool.tile([128, 32 * BCO], dt, tag="xT")
            for jc in range(n_inner):
                # 1. load natural-layout chunk [BC x N]
                xn = xn_pool.tile([BC, N], dt, tag="xn")
                nc.sync.dma_start(
                    xn[:], x_ap[cc * BCO + jc * BC:cc * BCO + (jc + 1) * BC, :]
                )
                for g in range(8):
                    # transpose output dtype must match input dtype
                    psT = psT_pool.tile([128, 4 * BC], dt, tag="psT")
                    for jj in range(4):
                        o = 4 * g + jj
                        nc.tensor.transpose(
                            psT[:, jj * BC:(jj + 1) * BC],
                            xn[:, o * 128:(o + 1) * 128],
                            identw[:],
                        )
                    # scatter the 4 transposed blocks to their o-slices
                    nc.any.tensor_copy(
                        xT[:].rearrange("p (o b) -> p o b", b=BCO)[
                            :, 4 * g:4 * g + 4, jc * BC:(jc + 1) * BC
                        ],
                        psT[:].rearrange("p (o b) -> p o b", o=4),
                    )

            # 3. pass A: y1[(o,i'), b] = sum_i A_o[i', i] xT[(o,i), b]
            #    y1 reuses the xT buffer (slice o dead once matmul o issued)
            y1 = xT
            for op in range(16):  # o-pairs -> one PSUM bank per pair
                psA = psA_pool.tile([128, 2 * BCO], f32, tag="psA")
                for jj in range(2):
                    o = 2 * op + jj
                    nc.tensor.matmul(
                        psA[:, jj * BCO:(jj + 1) * BCO],
                        ATw[:, o * 128:(o + 1) * 128],
                        xT[:, o * BCO:(o + 1) * BCO],
                        start=True,
                        stop=True,
                    )
                nc.any.tensor_copy(
                    y1[:, 2 * op * BCO:(2 * op + 2) * BCO], psA[:]
                )

            # 4. permute: z[(a*32+o), tau*BCO+b] = y1[4*tau+a, o*BCO+b]
            z = z_pool.tile([128, 32 * BCO], dt, tag="z")
            for tau in range(32):
                eng = nc.scalar if (tau % 2 == 0) else nc.sync
                eng.dma_start(
                    z[:, tau * BCO:(tau + 1) * BCO],
                    y1[4 * tau:4 * (tau + 1), :].rearrange("a (o b) -> a o b", b=BCO),
                )

            # 5. pass B (data as stationary operand) + 6. scatter-copy
            for jc in range(n_inner):
                outb = out_pool.tile([BC, N], dt, tag="outb")
                outb_re = outb[:].rearrange(
                    "p (op gg tt a) -> gg p tt op a", op=32, gg=8, tt=4, a=4
                )
                for g in range(8):
                    psB = psB_pool.tile([BC, 512], f32, tag="psB")
                    for tt in range(4):
                        tau = 4 * g + tt
                        nc.tensor.matmul(
                            psB[:, tt * 128:(tt + 1) * 128],
                            z[:, tau * BCO + jc * BC:tau * BCO + (jc + 1) * BC],
                            Rw[:, tau * 128:(tau + 1) * 128],
                            start=True,
                            stop=True,
                        )
                    nc.any.tensor_copy(
                        outb_re[g],
                        psB[:].rearrange("p (tt op a) -> p tt op a", tt=4, op=32, a=4),
                    )

                # 7. contiguous store
                nc.sync.dma_start(
                    y_ap[cc * BCO + jc * BC:cc * BCO + (jc + 1) * BC, :], outb[:]
                )

    nc.compile()
    return nc


_CACHE = {}


def _get_program():
    import concourse.mybir as mybir

    key = COMPUTE
    if key not in _CACHE:
        if COMPUTE == "fp16":
            _CACHE[key] = (_build_program(np.float16, mybir.dt.float16), np.float16)
        else:
            _CACHE[key] = (_build_program(np.float32, mybir.dt.float32), np.float32)
    return _CACHE[key]


def run(x, twiddle, trace=False, trace_kwargs=None):
    """Run the butterfly kernel on 8 cores. Returns (out, BassKernelResults)."""
    from concourse.bass_utils import run_bass_kernel_spmd

    nc, np_dt = _get_program()

    AT, R = _compose_matrices(twiddle)
    # [o, k, m] -> [k, o*128+m] (matches the SBUF weight layout)
    ATd = np.ascontiguousarray(AT.transpose(1, 0, 2).reshape(128, 32 * 128)).astype(np_dt)
    Rd = np.ascontiguousarray(R.transpose(1, 0, 2).reshape(128, 32 * 128)).astype(np_dt)
    identd = np.eye(128, dtype=np_dt)

    x = np.asarray(x)
    in_dtype = x.dtype
    xd = np.ascontiguousarray(x.astype(np_dt))

    in_maps = []
    for c in range(N_CORES):
        in_maps.append(
            {
                "x": xd[c * B_CORE:(c + 1) * B_CORE],
                "AT": ATd,
                "R": Rd,
                "ident": identd,
            }
        )

    res = run_bass_kernel_spmd(
        nc,
        in_maps,
        core_ids=list(range(N_CORES)),
        trace=trace,
        **(trace_kwargs or {}),
    )
    out = np.concatenate([r["y"] for r in res.results], axis=0)
    return out.astype(in_dtype), res


def kernel(x, twiddle):
    out, _ = run(x, twiddle)
    return out


# revision 13
# speedup vs baseline: 2.3898x; 1.0584x over previous
"""Butterfly multiply (n=4096, 12 stages, increasing stride) on 8 Trainium2
NeuronCores.

Math: the 12 butterfly stages factor into
  out = P^T-scatter( B-blockdiag @ P-permute( A-blockdiag @ x^T ) )
where stages 0..6 (strides 1..64) compose into 32 dense 128x128 matrices A_o
acting within 128-aligned blocks, and stages 7..11 (strides 128..2048) compose
into 128 dense 32x32 matrices C_i acting across blocks at fixed within-block
index.  Both are composed on the host from the (tiny) twiddle input; the heavy
data (x: 128 MiB) runs through two TensorEngine matmul passes per core.

Sharding: batch 8192 split across 8 cores (data parallel), twiddle-derived
matrices replicated.
"""

import os
import sys
import numpy as np

LOG_N = 12
N = 4096
BATCH = 8192
N_CORES = 8
B_CORE = BATCH // N_CORES  # 1024 rows per core

# compute dtype: "fp32" (safe, PE quarter-rate) or "fp16" (fast, ~1e-3 rel err)
COMPUTE = os.environ.get("BUTTERFLY_COMPUTE", "fp32")
BC = 128   # inner chunk (transpose / pass-B granularity)
BCO = 512 if COMPUTE == "fp16" else 256  # outer chunk (pass-A free dim, permute)


def _compose_matrices(twiddle):
    """Compose stages 0..6 -> A (32,128,128) and stages 7..11 -> C (128,32,32),
    in float64."""
    tw = np.asarray(twiddle)[0, 0].astype(np.float64)  # (12, 2048, 2, 2)

    A = np.zeros((32, 128, 128))
    A[:, np.arange(128), np.arange(128)] = 1.0
    for idx in range(7):
        s = 1 << idx
        Ar = A.reshape(32, 128 // (2 * s), 2, s, 128)  # (o, dl, k, j, i_in)
        o = np.arange(32)[:, None, None]
        dl = np.arange(128 // (2 * s))[None, :, None]
        j = np.arange(s)[None, None, :]
        m = (o * (64 // s) + dl) * s + j
        t = tw[idx, m]  # (32, dl, j, 2, 2)
        x0, x1 = Ar[:, :, 0], Ar[:, :, 1]
        new0 = t[..., 0, 0:1] * x0 + t[..., 0, 1:2] * x1
        new1 = t[..., 1, 0:1] * x0 + t[..., 1, 1:2] * x1
        A = np.stack([new0, new1], axis=2).reshape(32, 128, 128)

    C = np.zeros((128, 32, 32))
    C[:, np.arange(32), np.arange(32)] = 1.0
    for idx in range(7, 12):
        s = 1 << idx
        sp = s // 128
        Cr = C.reshape(128, 32 // (2 * sp), 2, sp, 32)  # (i, dl, k, ol, o_in)
        i = np.arange(128)[None, None, :]
        dl = np.arange(32 // (2 * sp))[:, None, None]
        ol = np.arange(sp)[None, :, None]
        m = dl * (128 * sp) + 128 * ol + i  # (dl, ol, i)
        t = np.moveaxis(tw[idx, m], 2, 0)  # (i, dl, ol, 2, 2)
        x0, x1 = Cr[:, :, 0], Cr[:, :, 1]
        new0 = t[..., 0, 0:1] * x0 + t[..., 0, 1:2] * x1
        new1 = t[..., 1, 0:1] * x0 + t[..., 1, 1:2] * x1
        C = np.stack([new0, new1], axis=2).reshape(128, 32, 32)

    # AT[o] = A[o].T  (lhsT layout for pass A)
    AT = np.ascontiguousarray(np.transpose(A, (0, 2, 1)))
    # R[tau][u=(a*32+o_in), v=(o_out*4+a)] = C[4*tau+a][o_out, o_in]
    R = np.zeros((32, 128, 128))
    for tau in range(32):
        for a in range(4):
            R[tau, a * 32:(a + 1) * 32, a::4] = C[4 * tau + a].T
    return AT, R


def _build_program(np_dt, mybir_dt, b_core=B_CORE):
    """Trace + compile the per-core Bass program. Returns nc."""
    import concourse.bacc as bacc
    import concourse.tile as tile
    import concourse.mybir as mybir
    from contextlib import ExitStack

    f32 = mybir.dt.float32
    dt = mybir_dt

    nc = bacc.Bacc(
        "TRN2",
        target_bir_lowering=False,
        debug=False,
        enable_asserts=False,
        num_devices=1,
    )
    x_ap = nc.dram_tensor("x", (b_core, N), dt, kind="ExternalInput").ap()
    # AT/R shipped pre-arranged as [k, o*128+m] so the load is a plain 2D copy
    at_ap = nc.dram_tensor("AT", (128, 32 * 128), dt, kind="ExternalInput").ap()
    r_ap = nc.dram_tensor("R", (128, 32 * 128), dt, kind="ExternalInput").ap()
    id_ap = nc.dram_tensor("ident", (128, 128), dt, kind="ExternalInput").ap()
    y_ap = nc.dram_tensor("y", (b_core, N), dt, kind="ExternalOutput").ap()

    n_outer = b_core // BCO
    n_inner = BCO // BC

    with tile.TileContext(nc) as tc, ExitStack() as ctx:
        wpool = ctx.enter_context(tc.tile_pool(name="weights", bufs=1))
        xn_pool = ctx.enter_context(tc.tile_pool(name="xn", bufs=3))
        # xT doubles as y1: pass A's copy for block o overwrites the xT slice
        # of block o right after the matmul that consumed it.
        xT_pool = ctx.enter_context(tc.tile_pool(name="xT", bufs=1))
        z_pool = ctx.enter_context(tc.tile_pool(name="z", bufs=2))
        out_pool = ctx.enter_context(tc.tile_pool(name="outb", bufs=3))
        psT_pool = ctx.enter_context(tc.tile_pool(name="psT", bufs=2, space="PSUM"))
        psA_pool = ctx.enter_context(tc.tile_pool(name="psA", bufs=2, space="PSUM"))
        psB_pool = ctx.enter_context(tc.tile_pool(name="psB", bufs=2, space="PSUM"))

        ATw = wpool.tile([128, 32 * 128], dt, tag="ATw")
        Rw = wpool.tile([128, 32 * 128], dt, tag="Rw")
        identw = wpool.tile([128, 128], dt, tag="identw")
        nc.sync.dma_start(ATw[:], at_ap)
        nc.sync.dma_start(Rw[:], r_ap)
        nc.sync.dma_start(identw[:], id_ap)

        for cc in range(n_outer):
            # 2. PE transpose to n-on-partitions + copy PSUM->SBUF
            #    xT free layout: o*BCO + jc*BC + b
            xT = xT_pool.tile([128, 32 * BCO], dt, tag="xT")
            for jc in range(n_inner):
                # 1. load natural-layout chunk [BC x N]
                xn = xn_pool.tile([BC, N], dt, tag="xn")
                nc.sync.dma_start(
                    xn[:], x_ap[cc * BCO + jc * BC:cc * BCO + (jc + 1) * BC, :]
                )
                for g in range(8):
                    # transpose output dtype must match input dtype
                    psT = psT_pool.tile([128, 4 * BC], dt, tag="psT")
                    for jj in range(4):
                        o = 4 * g + jj
                        nc.tensor.transpose(
                            psT[:, jj * BC:(jj + 1) * BC],
                            xn[:, o * 128:(o + 1) * 128],
                            identw[:],
                        )
                    # scatter the 4 transposed blocks to their o-slices
                    nc.any.tensor_copy(
                        xT[:].rearrange("p (o b) -> p o b", b=BCO)[
                            :, 4 * g:4 * g + 4, jc * BC:(jc + 1) * BC
                        ],
                        psT[:].rearrange("p (o b) -> p o b", o=4),
                    )

            # 3. pass A: y1[(o,i'), b] = sum_i A_o[i', i] xT[(o,i), b]
            #    y1 reuses the xT buffer (slice o dead once matmul o issued)
            y1 = xT
            n_sub = max(1, BCO // 512)  # fp32 matmul moving-dim cap is 512
            for o in range(32):
                psA = psA_pool.tile([128, BCO], f32, tag="psA")
                for ss in range(n_sub):
                    w = BCO // n_sub
                    nc.tensor.matmul(
                        psA[:, ss * w:(ss + 1) * w],
                        ATw[:, o * 128:(o + 1) * 128],
                        xT[:, o * BCO + ss * w:o * BCO + (ss + 1) * w],
                        start=True,
                        stop=True,
                    )
                nc.any.tensor_copy(y1[:, o * BCO:(o + 1) * BCO], psA[:])

            # 4. permute: z[(a*32+o), tau*BCO+b] = y1[4*tau+a, o*BCO+b]
            #    SWDGE (gpsimd) so descriptor-gen rides the otherwise-idle Q7s
            z = z_pool.tile([128, 32 * BCO], dt, tag="z")
            for tau in range(32):
                nc.gpsimd.dma_start(
                    z[:, tau * BCO:(tau + 1) * BCO],
                    y1[4 * tau:4 * (tau + 1), :].rearrange("a (o b) -> a o b", b=BCO),
                )

            # 5. pass B (data as stationary operand) + 6. scatter-copy
            for jc in range(n_inner):
                outb = out_pool.tile([BC, N], dt, tag="outb")
                outb_re = outb[:].rearrange(
                    "p (op gg tt a) -> gg p tt op a", op=32, gg=8, tt=4, a=4
                )
                for g in range(8):
                    psB = psB_pool.tile([BC, 512], f32, tag="psB")
                    for tt in range(4):
                        tau = 4 * g + tt
                        nc.tensor.matmul(
                            psB[:, tt * 128:(tt + 1) * 128],
                            z[:, tau * BCO + jc * BC:tau * BCO + (jc + 1) * BC],
                            Rw[:, tau * 128:(tau + 1) * 128],
                            start=True,
                            stop=True,
                        )
                    nc.any.tensor_copy(
                        outb_re[g],
                        psB[:].rearrange("p (tt op a) -> p tt op a", tt=4, op=32, a=4),
                    )

                # 7. contiguous store
                nc.sync.dma_start(
                    y_ap[cc * BCO + jc * BC:cc * BCO + (jc + 1) * BC, :], outb[:]
                )

    nc.compile()
    return nc


_CACHE = {}


def _get_program():
    import concourse.mybir as mybir

    key = COMPUTE
    if key not in _CACHE:
        if COMPUTE == "fp16":
            _CACHE[key] = (_build_program(np.float16, mybir.dt.float16), np.float16)
        else:
            _CACHE[key] = (_build_program(np.float32, mybir.dt.float32), np.float32)
    return _CACHE[key]


def run(x, twiddle, trace=False, trace_kwargs=None):
    """Run the butterfly kernel on 8 cores. Returns (out, BassKernelResults)."""
    from concourse.bass_utils import run_bass_kernel_spmd

    nc, np_dt = _get_program()

    AT, R = _compose_matrices(twiddle)
    # [o, k, m] -> [k, o*128+m] (matches the SBUF weight layout)
    ATd = np.ascontiguousarray(AT.transpose(1, 0, 2).reshape(128, 32 * 128)).astype(np_dt)
    Rd = np.ascontiguousarray(R.transpose(1, 0, 2).reshape(128, 32 * 128)).astype(np_dt)
    identd = np.eye(128, dtype=np_dt)

    x = np.asarray(x)
    in_dtype = x.dtype
    xd = np.ascontiguousarray(x.astype(np_dt))

    in_maps = []
    for c in range(N_CORES):
        in_maps.append(
            {
                "x": xd[c * B_CORE:(c + 1) * B_CORE],
                "AT": ATd,
                "R": Rd,
                "ident": identd,
            }
        )

    res = run_bass_kernel_spmd(
        nc,
        in_maps,
        core_ids=list(range(N_CORES)),
        trace=trace,
        **(trace_kwargs or {}),
    )
    out = np.concatenate([r["y"] for r in res.results], axis=0)
    return out.astype(in_dtype), res


def kernel(x, twiddle):
    out, _ = run(x, twiddle)
    return out
